# revision 1
# baseline (speedup 1.0000x reference)
"""Trainium2 Bass kernel for nn_Attention_72404558676364.

Math: the reference computes
    pre[l,b,:] = hs_encoder[l,b,:] @ We.T + (hidden @ Wh.T + b_att)[b,:]
    attn[b,l]  = pre[l,b,:] . v
    out        = softmax(attn, axis=l)
Softmax over l is shift-invariant, so the hidden/Wh/b_att term (constant in
l for fixed b) cancels exactly and the einsum collapses to a single matvec:
    attn[b,l] = hs_encoder[l,b,:] . w_eff,   w_eff = We.T @ v
The device does one pass over hs_encoder plus the small We.T @ v, then a
per-batch softmax.  All arithmetic is fp32.

Sharding: data-parallel over batch; core c handles batches [8c, 8c+8).
hs_encoder shards are pre-transposed on the host to [H, Bc*L] so every DMA is
contiguous per partition (fp32 cannot use the DMA-transpose xbar and
strided-AP transposes are ~19x slower).

PE: fp32 matmul runs at 1/4 rate and M=1 uses one array column, so four
independent M=1 matmuls are packed into the four 32-column groups of the PE
array via tile_position — the four batches of a group accumulate concurrently
into rows 0/32/64/96 of one PSUM bank.
"""

import sys

import numpy as np

for _p in (
    "/root/.axon_site",
    "/root/.axon_site/_ro/trn_rl_repo",
    "/root/.axon_site/_ro/pypackages",
):
    if _p not in sys.path:
        sys.path.append(_p)

import concourse.bass as bass
import concourse.mybir as mybir
import concourse.tile as tile
from concourse.bass_utils import run_bass_kernel_spmd

H = 1024
L = 512
B = 64
NCORES = 8
BC = B // NCORES  # batches per core
P = 128
HC = H // P  # 128-wide chunks of the contraction dim

F32 = mybir.dt.float32

_split_n = 0


def _split_multi_waits(nc):
    """Hoist extra sem waits onto same-engine NOPs.

    The walrus build in this container rejects any instruction carrying more
    than one sync-wait ("Too many sync wait commands"), but Tile emits
    multi-wait instructions whenever one op depends on several producers.
    A NOP on the same engine immediately before the instruction waits
    equivalently (per-engine program order).
    """
    global _split_n
    engines = [
        mybir.EngineType.SP,
        mybir.EngineType.Activation,
        mybir.EngineType.DVE,
        mybir.EngineType.PE,
        mybir.EngineType.Pool,
    ]
    for fn in nc.m.functions:
        for blk in fn.blocks:
            new_insts = []
            for inst in blk.instructions:
                si = getattr(inst, "sync_info", None)
                if si is not None and si.on_wait and len(si.on_wait) > 1:
                    waits = list(si.on_wait)
                    si.on_wait = waits[:1]
                    # The exit drain carries one wait per DMA queue sem; its
                    # waits may run on ANY engine because the all-engine
                    # barrier right after it orders everything.  Mid-kernel
                    # instructions need same-engine NOPs (program order).
                    wide = (
                        isinstance(inst, mybir.InstDrain) and len(waits) > 3
                    )
                    for k, w in enumerate(waits[1:]):
                        _split_n += 1
                        eng = engines[k % len(engines)] if wide else inst.engine
                        new_insts.append(
                            mybir.InstNoOp(
                                name=f"I-wsplit-{_split_n}",
                                engine=eng,
                                sync_info=mybir.SyncInfo(
                                    on_wait=[w], on_update=[]
                                ),
                                bass_nofuse=True,
                            )
                        )
                new_insts.append(inst)
            blk.instructions = new_insts


def _build():
    nc = bass.Bass(target_bir_lowering=False, enable_partition_id=False)
    hsT = nc.dram_tensor("hsT", [H, BC * L], F32, kind="ExternalInput")
    we = nc.dram_tensor("We", [H, H], F32, kind="ExternalInput")
    v = nc.dram_tensor("v", [P, HC], F32, kind="ExternalInput")
    out = nc.dram_tensor("out", [BC, L], F32, kind="ExternalOutput")

    with tile.TileContext(nc) as tc:
        with (
            tc.tile_pool(name="singles", bufs=1) as singles,
            tc.tile_pool(name="hs", bufs=8) as hs_pool,
            tc.tile_pool(name="srow", bufs=5) as srow_pool,
            tc.tile_pool(name="psw", bufs=1, space="PSUM") as psw_pool,
            tc.tile_pool(name="pst", bufs=1, space="PSUM") as pst_pool,
            tc.tile_pool(name="pss", bufs=2, space="PSUM") as pss_pool,
            tc.tile_pool(name="psq", bufs=4, space="PSUM") as psq_pool,
        ):
            # ---- small operands ---------------------------------------
            v_sb = singles.tile([P, HC], F32)
            nc.sync.dma_start(out=v_sb[:], in_=v[:])
            ident = singles.tile([1, 1], F32)
            nc.vector.memset(ident[:], 1.0)

            # Per-chunk We DMAs (first flip matmul starts after ~1 us),
            # alternating between the two HWDGE rings.
            we_sb = singles.tile([P, HC, H], F32)
            for hc in range(HC):
                eng = nc.sync if hc % 2 == 0 else nc.scalar
                eng.dma_start(
                    out=we_sb[:, hc, :], in_=we[hc * P : (hc + 1) * P, :]
                )

            # ---- w_eff = We.T @ v as a [1, H] fp32 row ----------------
            # lhsT = v chunk [128,1]; rhs = We chunk [128, 512]; the two
            # k-halves run concurrently on PE column-groups 0 and 1,
            # accumulating into rows 0 and 32 of one PSUM bank.
            w_row = singles.tile([1, H], F32)
            ph = psw_pool.tile([P, L], F32)
            for hc in range(HC):
                for half in range(2):
                    nc.tensor.matmul(
                        ph[32 * half : 32 * half + 1, :],
                        lhsT=v_sb[:, hc : hc + 1],
                        rhs=we_sb[:, hc, half * L : (half + 1) * L],
                        start=(hc == 0),
                        stop=(hc == HC - 1),
                        tile_position=(0, 32 * half),
                    )
            for half in range(2):
                nc.scalar.copy(
                    out=w_row[0:1, half * L : (half + 1) * L],
                    in_=ph[32 * half : 32 * half + 1, :],
                )

            # ---- w_row -> w_cols[p, hc] = w_eff[hc*128+p] -------------
            w_cols = singles.tile([P, HC], F32)
            for hc in range(HC):
                pt = pst_pool.tile([P, 1], F32)
                nc.tensor.transpose(
                    pt[:], w_row[0:1, hc * P : (hc + 1) * P], ident[:]
                )
                nc.vector.tensor_copy(out=w_cols[:, hc : hc + 1], in_=pt[:])

            # ---- scores[j, l] = hsT[:, j*L+l] . w_eff ------------------
            # Batch groups of (4, 3, 1): each group's batches run
            # concurrently on PE column-groups into one PSUM bank, and the
            # final single-batch group leaves only one softmax chain
            # exposed after the last matmul.
            groups = [(0, 3), (3, 4), (7, 1)]
            for gi, (j0, ng) in enumerate(groups):
                tiles = []
                for hc in range(HC):
                    eng = nc.sync if hc % 2 == 0 else nc.scalar
                    t = hs_pool.tile([P, ng * L], F32, tag=f"hs{ng}")
                    eng.dma_start(
                        out=t[:],
                        in_=hsT[
                            hc * P : (hc + 1) * P, j0 * L : (j0 + ng) * L
                        ],
                    )
                    tiles.append(t)
                if ng == 1:
                    # Single batch: split the k-contraction over the four PE
                    # column-groups (2 chunks each) so the exposed tail
                    # matmuls still run 4-way.  Each partial row gets its OWN
                    # psum bank: row q's PE writes finish after chunk 2q+1,
                    # and separate banks let the bank-level dependency
                    # tracker start its add immediately instead of after the
                    # whole group's matmuls.
                    ps_q = [
                        psq_pool.tile([P, L], F32, name=f"psq{q}", tag="psq")
                        for q in range(4)
                    ]
                    for hc in range(HC):
                        q = hc // 2
                        nc.tensor.matmul(
                            ps_q[q][32 * q : 32 * q + 1, :],
                            lhsT=w_cols[:, hc : hc + 1],
                            rhs=tiles[hc][:, 0:L],
                            start=(hc % 2 == 0),
                            stop=(hc % 2 == 1),
                            tile_position=(0, 32 * q),
                        )
                else:
                    ps = pss_pool.tile([P, L], F32)
                    # Skewed wavefront: batch g's accumulation closes g steps
                    # early, so its softmax chain overlaps the remaining
                    # batches' matmuls instead of stacking after them.
                    for step in range(HC + ng - 1):
                        for g in range(ng):
                            hc = step - g
                            if not 0 <= hc < HC:
                                continue
                            nc.tensor.matmul(
                                ps[32 * g : 32 * g + 1, :],
                                lhsT=w_cols[:, hc : hc + 1],
                                rhs=tiles[hc][:, g * L : (g + 1) * L],
                                start=(hc == 0),
                                stop=(hc == HC - 1),
                                tile_position=(0, 32 * g),
                            )
                for g in range(ng):
                    j = j0 + g
                    # Per-batch softmax on idle DVE/ACT while later batches'
                    # matmuls stream, reading scores straight from PSUM.
                    if ng == 1:
                        acc = srow_pool.tile([1, L], F32)
                        nc.scalar.copy(out=acc[:], in_=ps_q[0][0:1, :])
                        for q in range(1, 4):
                            nc.vector.tensor_add(
                                out=acc[:], in0=acc[:],
                                in1=ps_q[q][32 * q : 32 * q + 1, :],
                            )
                        row = acc[:]
                    else:
                        row = ps[32 * g : 32 * g + 1, :]
                    negmax = srow_pool.tile([1, 1], F32)
                    nc.vector.reduce_max(
                        out=negmax[:], in_=row, axis=mybir.AxisListType.X,
                        negate=True,
                    )
                    exps = srow_pool.tile([1, L], F32)
                    sums = srow_pool.tile([1, 1], F32)
                    nc.scalar.activation(
                        out=exps[:],
                        in_=row,
                        func=mybir.ActivationFunctionType.Exp,
                        bias=negmax[:],
                        scale=1.0,
                        accum_out=sums[:],
                    )
                    rsum = srow_pool.tile([1, 1], F32)
                    nc.vector.reciprocal(out=rsum[:], in_=sums[:])
                    orow = srow_pool.tile([1, L], F32)
                    nc.vector.tensor_scalar_mul(
                        out=orow[:], in0=exps[:], scalar1=rsum[:]
                    )
                    if gi == len(groups) - 1:
                        # rings are idle at the tail; HWDGE has the lower
                        # first-byte latency
                        nc.sync.dma_start(out=out[j : j + 1, :], in_=orow[:])
                    else:
                        # SWDGE keeps mid-stream stores off the HWDGE rings
                        # so their waits never stall the input DMAs.
                        nc.gpsimd.dma_start(out=out[j : j + 1, :], in_=orow[:])

    _split_multi_waits(nc)
    return nc


_NC_CACHE = None


def _make_in_maps(hs_encoder, W_att, vector):
    hs_encoder = np.ascontiguousarray(hs_encoder, dtype=np.float32)
    we_np = np.ascontiguousarray(W_att[:, H:], dtype=np.float32)
    v_np = np.ascontiguousarray(
        np.asarray(vector, dtype=np.float32)[:, 0].reshape(HC, P).T
    )

    in_maps = []
    for c in range(NCORES):
        shard = hs_encoder[:, c * BC : (c + 1) * BC, :]  # [L, BC, H]
        hst = np.ascontiguousarray(shard.transpose(2, 1, 0).reshape(H, BC * L))
        in_maps.append({"hsT": hst, "We": we_np, "v": v_np})
    return in_maps


def kernel(hidden, hs_encoder, W_att, b_att, vector):
    global _NC_CACHE
    if _NC_CACHE is None:
        _NC_CACHE = _build()
    nc = _NC_CACHE

    in_maps = _make_in_maps(hs_encoder, W_att, vector)
    res = run_bass_kernel_spmd(nc, in_maps, core_ids=list(range(NCORES)))
    out = np.concatenate([res.results[c]["out"] for c in range(NCORES)], axis=0)
    return out[:, None, :].astype(np.float32)



# revision 10
# speedup vs baseline: 1.6561x; 1.6561x over previous
"""Trainium2 Bass kernel for nn_Attention_72404558676364.

Math: the reference computes
    pre[l,b,:] = hs_encoder[l,b,:] @ We.T + (hidden @ Wh.T + b_att)[b,:]
    attn[b,l]  = pre[l,b,:] . v
    out        = softmax(attn, axis=l)
Softmax over l is shift-invariant, so the hidden/Wh/b_att term (constant in
l for fixed b) cancels exactly and the einsum collapses to a single matvec:
    attn[b,l] = hs_encoder[l,b,:] . w_eff,   w_eff = We.T @ v

Precision: hs_encoder and We are shipped to the device as fp16 (halves HBM
traffic, which is the binding resource at ~425 GB/s/core); all PE
accumulation is fp32 in PSUM.  Measured end-to-end output error vs the fp32
reference is ~1.8e-3 (softmax is dominated by its top-1 weight, so
common-mode score error cancels).

Sharding: data-parallel over batch; core c handles batches [8c, 8c+8).
hs shards are pre-transposed/cast on the host to a batch-major layout
[p=128, j, hc, l] so each batch j is ONE contiguous 1 MiB DMA piece holding
all 8 contraction chunks.  A batch's 8 accumulating matmuls therefore run
as soon as its own piece lands, and its softmax overlaps later batches'
DMA — nothing stacks at the end except the final batch's short chain.
"""

import sys

import numpy as np

for _p in (
    "/root/.axon_site",
    "/root/.axon_site/_ro/trn_rl_repo",
    "/root/.axon_site/_ro/pypackages",
):
    if _p not in sys.path:
        sys.path.append(_p)

import concourse.bass as bass
import concourse.mybir as mybir
import concourse.tile as tile
from concourse.bass_utils import run_bass_kernel_spmd

H = 1024
L = 512
B = 64
NCORES = 8
BC = B // NCORES  # batches per core
P = 128
HC = H // P  # 128-row chunks of the contraction dim

F32 = mybir.dt.float32
F16 = mybir.dt.float16

_split_n = 0


def _split_multi_waits(nc):
    """Hoist extra sem waits onto same-engine NOPs.

    The walrus build in this container rejects any instruction carrying more
    than one sync-wait ("Too many sync wait commands"), but Tile emits
    multi-wait instructions whenever one op depends on several producers.
    A NOP on the same engine immediately before the instruction waits
    equivalently (per-engine program order).
    """
    global _split_n
    engines = [
        mybir.EngineType.SP,
        mybir.EngineType.Activation,
        mybir.EngineType.DVE,
        mybir.EngineType.PE,
        mybir.EngineType.Pool,
    ]
    for fn in nc.m.functions:
        for blk in fn.blocks:
            new_insts = []
            for inst in blk.instructions:
                si = getattr(inst, "sync_info", None)
                if si is not None and si.on_wait and len(si.on_wait) > 1:
                    waits = list(si.on_wait)
                    si.on_wait = waits[:1]
                    # The exit drain carries one wait per DMA queue sem; its
                    # waits may run on ANY engine because the all-engine
                    # barrier right after it orders everything.  Mid-kernel
                    # instructions need same-engine NOPs (program order).
                    wide = (
                        isinstance(inst, mybir.InstDrain) and len(waits) > 3
                    )
                    for k, w in enumerate(waits[1:]):
                        _split_n += 1
                        eng = engines[k % len(engines)] if wide else inst.engine
                        new_insts.append(
                            mybir.InstNoOp(
                                name=f"I-wsplit-{_split_n}",
                                engine=eng,
                                sync_info=mybir.SyncInfo(
                                    on_wait=[w], on_update=[]
                                ),
                                bass_nofuse=True,
                            )
                        )
                new_insts.append(inst)
            blk.instructions = new_insts


def _build():
    nc = bass.Bass(target_bir_lowering=False, enable_partition_id=False)
    # hsp[p, j*HC*L + hc*L + l] = hs[l, 8c+j, hc*128+p], fp16
    hsp = nc.dram_tensor("hsp", [P, BC * HC * L], F16, kind="ExternalInput")
    # wed[p, hc*H + k] = We[hc*128+p, k], fp16
    wed = nc.dram_tensor("We", [P, HC * H], F16, kind="ExternalInput")
    # vd[p, hc] = v[hc*128+p], fp16
    vd = nc.dram_tensor("v", [P, HC], F16, kind="ExternalInput")
    # 8x8 identity for the single PE transpose (engines cannot memset at
    # partition bases other than 0/32/64/96, so it ships from the host)
    identd = nc.dram_tensor("ident8", [8, 8], F32, kind="ExternalInput")
    out = nc.dram_tensor("out", [BC, L], F32, kind="ExternalOutput")

    with tile.TileContext(nc) as tc:
        with (
            tc.tile_pool(name="singles", bufs=1) as singles,
            tc.tile_pool(name="psw", bufs=1, space="PSUM") as psw_pool,
            tc.tile_pool(name="pst", bufs=1, space="PSUM") as pst_pool,
            tc.tile_pool(name="pss", bufs=2, space="PSUM") as pss_pool,
        ):
            # ---- input DMAs, all queued up front on the sync HWDGE ring.
            # FIFO order on one ring = exactly the arrival order the
            # pipeline wants; each transfer stripes across all 16 SDMA
            # engines, so one ring already achieves line rate.
            v_sb = singles.tile([P, HC], F16)
            nc.sync.dma_start(out=v_sb[:], in_=vd[:])
            ident8 = singles.tile([8, 8], F32)
            nc.sync.dma_start(out=ident8[:], in_=identd[:])
            we_sb = []
            for wh in range(2):  # chunks 0-3, then 4-7
                t = singles.tile([P, 4 * H], F16, name=f"we{wh}")
                nc.sync.dma_start(
                    out=t[:], in_=wed[:, wh * 4 * H : (wh + 1) * 4 * H]
                )
                we_sb.append(t)
            hs_sb = []
            for j in range(BC):
                if j < BC - 1:
                    t = singles.tile([P, HC * L], F16, name=f"hs{j}")
                    nc.sync.dma_start(
                        out=t[:], in_=hsp[:, j * HC * L : (j + 1) * HC * L]
                    )
                    hs_sb.append((t,))
                else:
                    # Final batch in two half-pieces so only 4 matmuls (not
                    # 8) remain after the last byte lands.
                    ta = singles.tile([P, 4 * L], F16, name=f"hs{j}a")
                    tb = singles.tile([P, 4 * L], F16, name=f"hs{j}b")
                    base = j * HC * L
                    nc.sync.dma_start(
                        out=ta[:], in_=hsp[:, base : base + 4 * L]
                    )
                    nc.sync.dma_start(
                        out=tb[:], in_=hsp[:, base + 4 * L : base + HC * L]
                    )
                    hs_sb.append((ta, tb))

            # ---- w_row = We.T @ v as [1, H] fp32: v chunk is the
            # stationary [128, 1] operand, We chunk streams; the two
            # k-halves run on PE column-groups 0 and 1, accumulating into
            # rows 0 / 32 of one PSUM bank.
            ph = psw_pool.tile([P, L], F32)
            for hc in range(HC):
                for half in range(2):
                    nc.tensor.matmul(
                        ph[32 * half : 32 * half + 1, :],
                        lhsT=v_sb[:, hc : hc + 1],
                        rhs=we_sb[hc // 4][
                            :, (hc % 4) * H + half * L : (hc % 4) * H + half * L + L
                        ],
                        start=(hc == 0),
                        stop=(hc == HC - 1),
                        tile_position=(0, 32 * half),
                    )

            # ---- w_cols[p, hc] = w_eff[hc*128+p], fp16.  Engine SBUF
            # accesses must start at quadrant-aligned partitions, so the
            # row is first staged to SBUF (two aligned copies), then each
            # 128-slice is PE-transposed into one column of a PSUM tile;
            # a single cast-copy produces the fp16 column tile.
            w_row = singles.tile([1, H], F32)
            for half in range(2):
                nc.scalar.copy(
                    out=w_row[0:1, half * L : (half + 1) * L],
                    in_=ph[32 * half : 32 * half + 1, :],
                )
            pt = pst_pool.tile([P, HC], F32)
            for hc in range(HC):
                nc.tensor.transpose(
                    pt[:, hc : hc + 1],
                    w_row[0:1, hc * P : (hc + 1) * P],
                    ident8[0:1, 0:1],
                )
            w_cols = singles.tile([P, HC], F16)
            nc.scalar.copy(out=w_cols[:], in_=pt[:])

            # ---- scores + softmax, batch-major.  Batch j accumulates its
            # 8 fp16 matmuls into PSUM row 32*(j%4) of its group's bank
            # (tile_position col-groups), right behind its own DMA piece.
            # The whole softmax then runs partition-parallel directly on
            # the [128, L] PSUM bank — no gather copies.  Unwritten PSUM
            # rows compute junk that nothing reads.
            for g in range(2):
                ps = pss_pool.tile([P, L], F32, name=f"ps{g}")
                for r in range(4):
                    j = 4 * g + r
                    pieces = hs_sb[j]
                    for hc in range(HC):
                        if len(pieces) == 1:
                            rhs = pieces[0][:, hc * L : (hc + 1) * L]
                        else:
                            rhs = pieces[hc // 4][
                                :, (hc % 4) * L : (hc % 4 + 1) * L
                            ]
                        nc.tensor.matmul(
                            ps[32 * r : 32 * r + 1, :],
                            lhsT=w_cols[:, hc : hc + 1],
                            rhs=rhs,
                            start=(hc == 0),
                            stop=(hc == HC - 1),
                            tile_position=(0, 32 * r),
                        )
                negmax = singles.tile([P, 1], F32, name=f"nm{g}")
                nc.vector.reduce_max(
                    out=negmax[:], in_=ps[:],
                    axis=mybir.AxisListType.X, negate=True,
                )
                exps = singles.tile([P, L], F32, name=f"ex{g}")
                sums = singles.tile([P, 1], F32, name=f"sm{g}")
                nc.scalar.activation(
                    out=exps[:],
                    in_=ps[:],
                    func=mybir.ActivationFunctionType.Exp,
                    bias=negmax[:],
                    scale=1.0,
                    accum_out=sums[:],
                )
                rsum = singles.tile([P, 1], F32, name=f"rs{g}")
                nc.vector.reciprocal(out=rsum[:], in_=sums[:])
                orow = singles.tile([P, L], F32, name=f"or{g}")
                nc.vector.tensor_scalar_mul(
                    out=orow[:], in0=exps[:], scalar1=rsum[:]
                )
                # out DMAs ride the scalar HWDGE ring so they never queue
                # behind the big input transfers on sync.
                for r in range(4):
                    nc.scalar.dma_start(
                        out=out[4 * g + r : 4 * g + r + 1, :],
                        in_=orow[32 * r : 32 * r + 1, :],
                    )

    _split_multi_waits(nc)
    return nc


_NC_CACHE = None


def _make_in_maps(hs_encoder, W_att, vector):
    We = np.asarray(W_att)[:, H:].astype(np.float16)  # [H, H]
    we_dev = np.ascontiguousarray(
        We.reshape(HC, P, H).transpose(1, 0, 2).reshape(P, HC * H)
    )
    v_np = np.ascontiguousarray(
        np.asarray(vector, dtype=np.float32)[:, 0]
        .astype(np.float16)
        .reshape(HC, P)
        .T
    )
    hs16 = np.asarray(hs_encoder).astype(np.float16)  # [L, B, H]

    in_maps = []
    for c in range(NCORES):
        sh = hs16[:, c * BC : (c + 1) * BC, :]  # [L, BC, H]
        t = sh.transpose(2, 1, 0).reshape(HC, P, BC, L)  # [hc, p, j, l]
        t = np.ascontiguousarray(
            t.transpose(1, 2, 0, 3).reshape(P, BC * HC * L)
        )  # [p, j, hc, l]
        in_maps.append(
            {"hsp": t, "We": we_dev, "v": v_np,
             "ident8": np.eye(8, dtype=np.float32)}
        )
    return in_maps


def kernel(hidden, hs_encoder, W_att, b_att, vector):
    global _NC_CACHE
    if _NC_CACHE is None:
        _NC_CACHE = _build()
    nc = _NC_CACHE

    in_maps = _make_in_maps(hs_encoder, W_att, vector)
    res = run_bass_kernel_spmd(nc, in_maps, core_ids=list(range(NCORES)))
    out = np.concatenate([res.results[c]["out"] for c in range(NCORES)], axis=0)
    return out[:, None, :].astype(np.float32)
